# revision 12
# baseline (speedup 1.0000x reference)
"""Multi-head causal attention on 8 trn2 NeuronCores.

Sharding: data-parallel over batch (2) x tensor-parallel over heads (4 per
core, Megatron-style column-split QKV / row-split output projection).
Per-core partial outputs are summed on the host (+ output bias).
"""

import sys

sys.path.insert(0, "/opt/trn_rl_repo")

import ml_dtypes
import numpy as np

import concourse.bass as bass  # noqa: F401  (import keeps bass registered)
import concourse.tile as tile
from concourse import bacc, mybir

BF16 = mybir.dt.bfloat16
F32 = mybir.dt.float32
AF = mybir.ActivationFunctionType

N = 2048  # sequence length
D = 1024  # model dim
NC = 8  # cores


def build_nc(variant="full", loop=1):
    """Build the (SPMD) Bass program run identically on all 8 cores.

    variant: "full" | "nopb" (skip partition_broadcast, copy unnormalized ctx)
    loop: repeat the whole body N times inside the NEFF (timing harness).
    """
    nc = bacc.Bacc("TRN2", target_bir_lowering=False, debug=False, num_devices=NC)

    xT = nc.declare_dram_parameter("xT", [8, 128, N], BF16, isOutput=False)
    wq = nc.declare_dram_parameter("wq", [8, 128, 256], BF16, isOutput=False)
    wk = nc.declare_dram_parameter("wk", [8, 128, 256], BF16, isOutput=False)
    wv = nc.declare_dram_parameter("wv", [8, 128, 260], BF16, isOutput=False)
    bqp = nc.declare_dram_parameter("bq", [128, 2], F32, isOutput=False)
    bkp = nc.declare_dram_parameter("bk", [128, 2], F32, isOutput=False)
    bvcp = nc.declare_dram_parameter("bvc", [1, 260], BF16, isOutput=False)
    wo = nc.declare_dram_parameter("wo", [128, 2, 1024], BF16, isOutput=False)
    maskp = nc.declare_dram_parameter("mask", [128, 128], BF16, isOutput=False)
    outp = nc.declare_dram_parameter("out", [N, 1024], F32, isOutput=True)

    with tile.TileContext(nc) as tc:
        with tc.tile_pool(name="singles", bufs=1) as singles:
            xt_sb = singles.tile([128, 8, N], BF16)
            wq_sb = singles.tile([128, 8, 256], BF16)
            wk_sb = singles.tile([128, 8, 256], BF16)
            wv_sb = singles.tile([128, 8, 260], BF16)
            bq_sb = singles.tile([128, 2], F32)
            bk_sb = singles.tile([128, 2], F32)
            bvc_sb = singles.tile([1, 260], BF16)
            wo_sb = singles.tile([128, 2, 1024], BF16)
            mask_sb = singles.tile([128, 128], BF16)
            ones_sb = singles.tile([1, 128], BF16)
            qT_sb = singles.tile([128, 2, N], BF16)
            kT_sb = singles.tile([128, 2, N], BF16)
            vc_sb = singles.tile([128, 16, 260], BF16)
            ctxn_sb = singles.tile([128, 2, N], BF16)

            def _dma_in():
                nc.vector.memset(ones_sb[:, :], 1.0)
                # weights on the SWDGE path, activations on HWDGE — parallel
                # issue queues; one large strided DMA per tensor.
                nc.gpsimd.dma_start(
                    out=wq_sb[:, :, :], in_=wq[:, :, :].rearrange("k p n -> p k n")
                )
                nc.gpsimd.dma_start(
                    out=wk_sb[:, :, :], in_=wk[:, :, :].rearrange("k p n -> p k n")
                )
                nc.gpsimd.dma_start(
                    out=wv_sb[:, :, :], in_=wv[:, :, :].rearrange("k p n -> p k n")
                )
                nc.gpsimd.dma_start(out=bq_sb[:, :], in_=bqp[:, :])
                nc.gpsimd.dma_start(out=bk_sb[:, :], in_=bkp[:, :])
                nc.gpsimd.dma_start(out=bvc_sb[:, :], in_=bvcp[:, :])
                nc.gpsimd.dma_start(out=wo_sb[:, :, :], in_=wo[:, :, :])
                nc.gpsimd.dma_start(out=mask_sb[:, :], in_=maskp[:, :])
                nc.sync.dma_start(out=xt_sb[:, 0, :], in_=xT[0])
                nc.sync.dma_start(out=xt_sb[:, 1, :], in_=xT[1])
                for half in range(3):
                    k0 = 2 * half + 2
                    nc.sync.dma_start(
                        out=xt_sb[:, k0 : k0 + 2, :],
                        in_=xT[k0 : k0 + 2, :, :].rearrange("k p n -> p k n"),
                    )

            def _qk_proj(misc_ps, c):
                for w_sb, b_sb, o_sb in (
                    (wq_sb, bq_sb, qT_sb),
                    (wk_sb, bk_sb, kT_sb),
                ):
                    for I in range(4):
                        ps = misc_ps.tile([128, 1024], F32, tag="sc", name="qkps")
                        for kc in range(8):
                            nc.tensor.matmul(
                                ps[:, :512],
                                lhsT=w_sb[:, kc, 128 * c : 128 * (c + 1)],
                                rhs=xt_sb[:, kc, 512 * I : 512 * (I + 1)],
                                start=(kc == 0),
                                stop=(kc == 7),
                            )
                        nc.vector.tensor_scalar_add(
                            o_sb[:, c, 512 * I : 512 * (I + 1)],
                            ps[:, :512],
                            b_sb[:, c : c + 1],
                        )

            def _v_proj(misc_ps):
                for J in range(16):
                    ps = misc_ps.tile([128, 1024], F32, tag="sc", name="vps")
                    for kc in range(8):
                        nc.tensor.matmul(
                            ps[:, :260],
                            lhsT=xt_sb[:, kc, 128 * J : 128 * (J + 1)],
                            rhs=wv_sb[:, kc, :],
                            start=(kc == 0),
                            stop=False,
                        )
                    nc.tensor.matmul(
                        ps[:, :260],
                        lhsT=ones_sb[:, :],
                        rhs=bvc_sb[:, :],
                        start=False,
                        stop=True,
                    )
                    nc.vector.tensor_copy(out=vc_sb[:, J, :], in_=ps[:, :260])

            def _norm_chunk(znp, h, I, ctx_tile):
                c, po = h // 2, 64 * (h % 2)
                if variant == "nopb":
                    nc.vector.tensor_copy(
                        out=ctxn_sb[po : po + 64, c, 512 * I : 512 * (I + 1)],
                        in_=ctx_tile[0:64, :],
                    )
                    return
                zr = znp.tile([1, 512], F32, tag="zr", name="zr")
                nc.vector.reciprocal(zr[:, :], ctx_tile[64:65, :])
                zb = znp.tile([64, 512], F32, tag="zb", name="zb")
                nc.gpsimd.partition_broadcast(zb[:, :], zr[:, :], channels=64)
                nc.vector.tensor_mul(
                    ctxn_sb[po : po + 64, c, 512 * I : 512 * (I + 1)],
                    ctx_tile[0:64, :],
                    zb[:, :],
                )

            def _attn_pair(misc_ps, ctxp, ptp, znp, p):
                """Heads (2p, 2p+1) together: even head at partitions 0-63,
                odd at 64-127 -> row-disjoint tile_positions let the PE run
                both K=64 score matmuls concurrently."""
                c = p
                # two i-half passes bound PSUM: 2 score tiles + 4 ctx banks
                for phase in range(2):
                    i0, i1 = 1024 * phase, 1024 * (phase + 1)
                    ctx_t = {
                        (hh, I2): ctxp.tile(
                            [65, 512], F32,
                            name=f"ctx{hh}{I2}", tag=f"ctx{hh}{I2}",
                        )
                        for hh in range(2)
                        for I2 in range(2)
                    }
                    for J in range(8 * phase + 8):
                        gs0 = max(i0, 128 * J)
                        L = i1 - gs0
                        pts = []
                        for hh in range(2):
                            po = 64 * hh
                            ps = misc_ps.tile(
                                [128, 1024], F32, tag="sc", name="scps"
                            )
                            pt = ptp.tile([128, 1024], BF16, tag="pt", name="pt")
                            pts.append(pt)
                            for s in range(0, L, 512):
                                sw = min(512, L - s)
                                nc.tensor.matmul(
                                    ps[:, s : s + sw],
                                    lhsT=kT_sb[
                                        po : po + 64, c, 128 * J : 128 * (J + 1)
                                    ],
                                    rhs=qT_sb[po : po + 64, c, gs0 + s : gs0 + s + sw],
                                    start=True,
                                    stop=True,
                                )
                            nc.scalar.activation(
                                pt[:, :L], ps[:, :L], AF.Exp, scale=0.125
                            )
                            if J >= 8 * phase:  # diagonal block in range
                                nc.vector.tensor_mul(
                                    pt[:, :128], pt[:, :128], mask_sb[:, :]
                                )
                        for hh in range(2):
                            h = 2 * p + hh
                            for I2 in range(2):
                                I = 2 * phase + I2
                                if J > 4 * I + 3:
                                    continue
                                gs = max(512 * I, 128 * J)
                                ge = 512 * (I + 1)
                                nc.tensor.matmul(
                                    ctx_t[(hh, I2)][:, gs - 512 * I : ge - 512 * I],
                                    lhsT=vc_sb[:, J, 65 * h : 65 * h + 65],
                                    rhs=pts[hh][:, gs - gs0 : ge - gs0],
                                    start=(J == 0),
                                    stop=(J == 4 * I + 3),
                                )
                                if J == 4 * I + 3:
                                    _norm_chunk(znp, h, I, ctx_t[(hh, I2)])

            def _final(misc_ps, osb):
                for t in range(16):
                    for oc in range(2):
                        ps = misc_ps.tile([128, 1024], F32, tag="sc", name="fps")
                        for a in range(2):
                            nc.tensor.matmul(
                                ps[:, :512],
                                lhsT=ctxn_sb[:, a, 128 * t : 128 * (t + 1)],
                                rhs=wo_sb[:, a, 512 * oc : 512 * (oc + 1)],
                                start=(a == 0),
                                stop=(a == 1),
                            )
                        ot = osb.tile([128, 512], F32, tag="o", name="ot")
                        if (t + oc) % 2 == 0:
                            nc.vector.tensor_copy(out=ot[:, :], in_=ps[:, :512])
                        else:
                            nc.scalar.copy(out=ot[:, :], in_=ps[:, :512])
                        nc.sync.dma_start(
                            out=outp[
                                128 * t : 128 * (t + 1),
                                512 * oc : 512 * (oc + 1),
                            ],
                            in_=ot[:, :],
                        )

            def _iter():
                with tc.tile_pool(name="misc_ps", bufs=2, space="PSUM") as misc_ps, \
                     tc.tile_pool(name="ctx_ps", bufs=1, space="PSUM") as ctxp, \
                     tc.tile_pool(name="pt", bufs=4) as ptp, \
                     tc.tile_pool(name="zn", bufs=2) as znp, \
                     tc.tile_pool(name="osb", bufs=4) as osb:
                    _dma_in()
                    _qk_proj(misc_ps, 0)
                    _v_proj(misc_ps)
                    _attn_pair(misc_ps, ctxp, ptp, znp, 0)
                    _qk_proj(misc_ps, 1)
                    _attn_pair(misc_ps, ctxp, ptp, znp, 1)
                    _final(misc_ps, osb)

            if loop == 1:
                _iter()
            else:
                with tc.For_i(0, loop, 1):
                    _iter()

    nc.compile()
    return nc


class _Runner:
    """Jitted PJRT executor for the SPMD program (built once per process)."""

    def __init__(self, nc):
        import jax
        from jax.experimental.shard_map import shard_map
        from jax.sharding import Mesh, NamedSharding, PartitionSpec

        from concourse.bass2jax import (
            _bass_exec_p,
            install_neuronx_cc_hook,
            partition_id_tensor,
        )

        install_neuronx_cc_hook()
        self.nc = nc
        self.jax = jax

        in_names, out_names, out_avals = [], [], []
        partition_name = (
            nc.partition_id_tensor.name if nc.partition_id_tensor else None
        )
        for alloc in nc.m.functions[0].allocations:
            if not isinstance(alloc, mybir.MemoryLocationSet):
                continue
            name = alloc.memorylocations[0].name
            if alloc.kind == "ExternalInput":
                if name != partition_name:
                    in_names.append(name)
            elif alloc.kind == "ExternalOutput":
                out_names.append(name)
                out_avals.append(
                    jax.core.ShapedArray(
                        tuple(alloc.tensor_shape), mybir.dt.np(alloc.dtype)
                    )
                )
        self.in_names = list(in_names)
        self.out_names = out_names
        self.out_avals = out_avals
        n_params = len(in_names)
        n_outs = len(out_names)
        all_names = in_names + out_names
        if partition_name is not None:
            all_names = all_names + [partition_name]

        def _body(*args):
            operands = list(args)
            if partition_name is not None:
                operands.append(partition_id_tensor())
            return tuple(
                _bass_exec_p.bind(
                    *operands,
                    out_avals=tuple(out_avals),
                    in_names=tuple(all_names),
                    out_names=tuple(out_names),
                    lowering_input_output_aliases=(),
                    sim_require_finite=True,
                    sim_require_nnan=True,
                    nc=nc,
                )
            )

        devices = jax.devices()[:NC]
        self.mesh = Mesh(np.asarray(devices), ("core",))
        in_specs = (PartitionSpec("core"),) * (n_params + n_outs)
        out_specs = (PartitionSpec("core"),) * n_outs
        self.fn = jax.jit(
            shard_map(
                _body,
                mesh=self.mesh,
                in_specs=in_specs,
                out_specs=out_specs,
                check_rep=False,
            ),
            keep_unused=True,
        )
        self.sharding = NamedSharding(self.mesh, PartitionSpec("core"))

    def prep(self, in_maps):
        """Concatenate per-core inputs along axis 0 and device_put."""
        arrs = []
        for name in self.in_names:
            arrs.append(np.concatenate([m[name] for m in in_maps], axis=0))
        for av in self.out_avals:
            arrs.append(np.zeros((NC * av.shape[0], *av.shape[1:]), av.dtype))
        return [self.jax.device_put(a, self.sharding) for a in arrs]

    def run(self, dev_args):
        out = self.fn(*dev_args)
        self.jax.block_until_ready(out)
        return out

    def run_async(self, dev_args):
        return self.fn(*dev_args)

    def unpack(self, out):
        res = []
        for c in range(NC):
            res.append(
                {
                    name: np.asarray(out[i]).reshape(NC, *self.out_avals[i].shape)[c]
                    for i, name in enumerate(self.out_names)
                }
            )
        return res


_RUNNER = None


def _get_runner():
    global _RUNNER
    if _RUNNER is None:
        _RUNNER = _Runner(build_nc())
    return _RUNNER


def make_in_maps(x, Wq, bq, Wk, bk, Wv, bv, Wo, bo):
    bf = ml_dtypes.bfloat16
    f32 = np.float32
    x = np.asarray(x, f32)
    mask = np.ascontiguousarray(np.triu(np.ones((128, 128), f32))).astype(bf)
    in_maps = []
    for core in range(NC):
        b, g = core // 4, core % 4
        sl = slice(256 * g, 256 * (g + 1))
        wv_cat = np.zeros((D, 260), f32)
        bv_cat = np.zeros((1, 260), f32)
        for h in range(4):
            col = 256 * g + 64 * h
            wv_cat[:, 65 * h : 65 * h + 64] = Wv[:, col : col + 64]
            bv_cat[0, 65 * h : 65 * h + 64] = bv[col : col + 64]
            bv_cat[0, 65 * h + 64] = 1.0
        in_maps.append(
            {
                "xT": np.ascontiguousarray(x[b].T).reshape(8, 128, N).astype(bf),
                "wq": np.ascontiguousarray(Wq[:, sl]).reshape(8, 128, 256).astype(bf),
                "wk": np.ascontiguousarray(Wk[:, sl]).reshape(8, 128, 256).astype(bf),
                "wv": wv_cat.reshape(8, 128, 260).astype(bf),
                "bq": np.ascontiguousarray(np.asarray(bq, f32)[sl].reshape(2, 128).T),
                "bk": np.ascontiguousarray(np.asarray(bk, f32)[sl].reshape(2, 128).T),
                "bvc": bv_cat.astype(bf),
                "wo": np.ascontiguousarray(
                    np.asarray(Wo, f32)[sl].reshape(2, 128, 1024).transpose(1, 0, 2)
                ).astype(bf),
                "mask": mask,
            }
        )
    return in_maps


def combine(results, bo):
    out = np.zeros((2, N, D), np.float32)
    for core in range(NC):
        out[core // 4] += results[core]["out"]
    out += np.asarray(bo, np.float32)[None, None, :]
    return out


def kernel(x, Wq, bq, Wk, bk, Wv, bv, Wo, bo):
    runner = _get_runner()
    in_maps = make_in_maps(x, Wq, bq, Wk, bk, Wv, bv, Wo, bo)
    dev_args = runner.prep(in_maps)
    results = runner.unpack(runner.run(dev_args))
    return combine(results, bo)


# revision 20
# speedup vs baseline: 1.0377x; 1.0377x over previous
"""Multi-head causal attention on 8 trn2 NeuronCores.

Sharding: data-parallel over batch (2) x tensor-parallel over heads (4 per
core, Megatron-style column-split QKV / row-split output projection).
Per-core partial outputs are summed on the host (+ output bias).
"""

import sys

sys.path.insert(0, "/opt/trn_rl_repo")

import ml_dtypes
import numpy as np

import concourse.bass as bass  # noqa: F401  (import keeps bass registered)
import concourse.tile as tile
from concourse import bacc, mybir

BF16 = mybir.dt.bfloat16
F32 = mybir.dt.float32
AF = mybir.ActivationFunctionType

N = 2048  # sequence length
D = 1024  # model dim
NC = 8  # cores


def build_nc(variant="full", loop=1, phases="full"):
    """Build the (SPMD) Bass program run identically on all 8 cores.

    variant: "full" | "nopb" (skip partition_broadcast, copy unnormalized ctx)
    loop: repeat the whole body N times inside the NEFF (timing harness).
    phases: "dma" | "proj" | "attn" | "full" — truncate after that phase
        (debug builds; partial results sunk to the output tensor).
    """
    nc = bacc.Bacc("TRN2", target_bir_lowering=False, debug=False, num_devices=NC)

    xT = nc.declare_dram_parameter("xT", [8, 128, N], BF16, isOutput=False)
    wq = nc.declare_dram_parameter("wq", [8, 128, 256], BF16, isOutput=False)
    wk = nc.declare_dram_parameter("wk", [8, 128, 256], BF16, isOutput=False)
    wv = nc.declare_dram_parameter("wv", [8, 128, 260], BF16, isOutput=False)
    bqp = nc.declare_dram_parameter("bq", [128, 2], F32, isOutput=False)
    bkp = nc.declare_dram_parameter("bk", [128, 2], F32, isOutput=False)
    bvcp = nc.declare_dram_parameter("bvc", [1, 260], BF16, isOutput=False)
    wo = nc.declare_dram_parameter("wo", [128, 2, 1024], BF16, isOutput=False)
    maskp = nc.declare_dram_parameter("mask", [128, 128], BF16, isOutput=False)
    outp = nc.declare_dram_parameter("out", [N, 1024], F32, isOutput=True)

    with tile.TileContext(nc) as tc:
        with tc.tile_pool(name="singles", bufs=1) as singles:
            xt_sb = singles.tile([128, 8, N], BF16)
            wq_sb = singles.tile([128, 8, 256], BF16)
            wk_sb = singles.tile([128, 8, 256], BF16)
            wv_sb = singles.tile([128, 8, 260], BF16)
            bq_sb = singles.tile([128, 2], F32)
            bk_sb = singles.tile([128, 2], F32)
            bvc_sb = singles.tile([1, 260], BF16)
            wo_sb = singles.tile([128, 2, 1024], BF16)
            mask_sb = singles.tile([128, 128], BF16)
            ones_sb = singles.tile([1, 128], BF16)
            qT_sb = singles.tile([128, 2, N], BF16)
            kT_sb = singles.tile([128, 2, N], BF16)
            vc_sb = singles.tile([128, 16, 260], BF16)
            ctxn_sb = singles.tile([128, 2, N], BF16)

            def _dma_in():
                nc.vector.memset(ones_sb[:, :], 1.0)
                # weights on the SWDGE path, activations on HWDGE — parallel
                # issue queues; one large strided DMA per tensor.
                nc.gpsimd.dma_start(
                    out=wq_sb[:, :, :], in_=wq[:, :, :].rearrange("k p n -> p k n")
                )
                nc.gpsimd.dma_start(
                    out=wk_sb[:, :, :], in_=wk[:, :, :].rearrange("k p n -> p k n")
                )
                nc.gpsimd.dma_start(
                    out=wv_sb[:, :, :], in_=wv[:, :, :].rearrange("k p n -> p k n")
                )
                nc.gpsimd.dma_start(out=bq_sb[:, :], in_=bqp[:, :])
                nc.gpsimd.dma_start(out=bk_sb[:, :], in_=bkp[:, :])
                nc.gpsimd.dma_start(out=bvc_sb[:, :], in_=bvcp[:, :])
                nc.gpsimd.dma_start(out=wo_sb[:, :, :], in_=wo[:, :, :])
                nc.gpsimd.dma_start(out=mask_sb[:, :], in_=maskp[:, :])
                nc.sync.dma_start(out=xt_sb[:, 0, :], in_=xT[0])
                nc.sync.dma_start(out=xt_sb[:, 1, :], in_=xT[1])
                for half in range(3):
                    k0 = 2 * half + 2
                    nc.sync.dma_start(
                        out=xt_sb[:, k0 : k0 + 2, :],
                        in_=xT[k0 : k0 + 2, :, :].rearrange("k p n -> p k n"),
                    )

            def _qk_unit(misc_ps, w_sb, b_sb, o_sb, c, I):
                ps = misc_ps.tile([128, 1024], F32, tag="sc", name="qkps")
                for kc in range(8):
                    nc.tensor.matmul(
                        ps[:, :512],
                        lhsT=w_sb[:, kc, 128 * c : 128 * (c + 1)],
                        rhs=xt_sb[:, kc, 512 * I : 512 * (I + 1)],
                        start=(kc == 0),
                        stop=(kc == 7),
                    )
                nc.vector.tensor_scalar_add(
                    o_sb[:, c, 512 * I : 512 * (I + 1)],
                    ps[:, :512],
                    b_sb[:, c : c + 1],
                )

            def _qk_units(misc_ps, c):
                for w_sb, b_sb, o_sb in (
                    (wq_sb, bq_sb, qT_sb),
                    (wk_sb, bk_sb, kT_sb),
                ):
                    for I in range(4):
                        yield lambda w=w_sb, b=b_sb, o=o_sb, i=I: _qk_unit(
                            misc_ps, w, b, o, c, i
                        )

            def _qk_proj(misc_ps, c):
                for u in _qk_units(misc_ps, c):
                    u()

            def _v_proj(misc_ps):
                for J in range(16):
                    ps = misc_ps.tile([128, 1024], F32, tag="sc", name="vps")
                    for kc in range(8):
                        nc.tensor.matmul(
                            ps[:, :260],
                            lhsT=xt_sb[:, kc, 128 * J : 128 * (J + 1)],
                            rhs=wv_sb[:, kc, :],
                            start=(kc == 0),
                            stop=False,
                        )
                    nc.tensor.matmul(
                        ps[:, :260],
                        lhsT=ones_sb[:, :],
                        rhs=bvc_sb[:, :],
                        start=False,
                        stop=True,
                    )
                    nc.vector.tensor_copy(out=vc_sb[:, J, :], in_=ps[:, :260])

            def _norm_chunk(znp, h, I, ctx_tile):
                c, po = h // 2, 64 * (h % 2)
                if variant == "nopb":
                    nc.vector.tensor_copy(
                        out=ctxn_sb[po : po + 64, c, 512 * I : 512 * (I + 1)],
                        in_=ctx_tile[0:64, :],
                    )
                    return
                zr = znp.tile([1, 512], F32, tag="zr", name="zr")
                nc.vector.reciprocal(zr[:, :], ctx_tile[64:65, :])
                zb = znp.tile([64, 512], F32, tag="zb", name="zb")
                nc.gpsimd.partition_broadcast(zb[:, :], zr[:, :], channels=64)
                nc.vector.tensor_mul(
                    ctxn_sb[po : po + 64, c, 512 * I : 512 * (I + 1)],
                    ctx_tile[0:64, :],
                    zb[:, :],
                )

            def _attn_pair(misc_ps, ctxp, ptp, znp, p, hook=None):
                """Heads (2p, 2p+1) together: even head at partitions 0-63,
                odd at 64-127 -> row-disjoint tile_positions let the PE run
                both K=64 score matmuls concurrently.  `hook(phase, J)` emits
                interleaved filler work after each J iteration so the
                in-order PE queue has something to chew on while ACT runs
                the exp chain."""
                c = p
                # two i-half passes bound PSUM: 2 score tiles + 4 ctx banks
                for phase in range(2):
                    i0, i1 = 1024 * phase, 1024 * (phase + 1)
                    ctx_t = {
                        (hh, I2): ctxp.tile(
                            [65, 512], F32,
                            name=f"ctx{hh}{I2}", tag=f"ctx{hh}{I2}",
                        )
                        for hh in range(2)
                        for I2 in range(2)
                    }
                    for J in range(8 * phase + 8):
                        gs0 = max(i0, 128 * J)
                        L = i1 - gs0
                        pts = []
                        for hh in range(2):
                            po = 64 * hh
                            ps = misc_ps.tile(
                                [128, 1024], F32, tag="sc", name="scps"
                            )
                            pt = ptp.tile([128, 1024], BF16, tag="pt", name="pt")
                            pts.append(pt)
                            for s in range(0, L, 512):
                                sw = min(512, L - s)
                                nc.tensor.matmul(
                                    ps[:, s : s + sw],
                                    lhsT=kT_sb[
                                        po : po + 64, c, 128 * J : 128 * (J + 1)
                                    ],
                                    rhs=qT_sb[po : po + 64, c, gs0 + s : gs0 + s + sw],
                                    start=True,
                                    stop=True,
                                )
                            nc.scalar.activation(
                                pt[:, :L], ps[:, :L], AF.Exp, scale=0.125
                            )
                            if J >= 8 * phase and variant != "nomask":
                                nc.vector.tensor_mul(
                                    pt[:, :128], pt[:, :128], mask_sb[:, :]
                                )
                        if variant == "noctx":
                            for hh in range(2):
                                nc.vector.tensor_copy(
                                    out=ctxn_sb[0:128, c, J : J + 1],
                                    in_=pts[hh][:, 0:1],
                                )
                            continue
                        for hh in range(2):
                            h = 2 * p + hh
                            for I2 in range(2):
                                I = 2 * phase + I2
                                if J > 4 * I + 3:
                                    continue
                                gs = max(512 * I, 128 * J)
                                ge = 512 * (I + 1)
                                nc.tensor.matmul(
                                    ctx_t[(hh, I2)][:, gs - 512 * I : ge - 512 * I],
                                    lhsT=vc_sb[:, J, 65 * h : 65 * h + 65],
                                    rhs=pts[hh][:, gs - gs0 : ge - gs0],
                                    start=(J == 0),
                                    stop=(J == 4 * I + 3),
                                )
                                if J == 4 * I + 3:
                                    _norm_chunk(znp, h, I, ctx_t[(hh, I2)])
                        if hook is not None:
                            hook(phase, J)

            def _final_unit(misc_ps, osb, t, oc):
                ps = misc_ps.tile([128, 1024], F32, tag="sc", name="fps")
                for a in range(2):
                    nc.tensor.matmul(
                        ps[:, :512],
                        lhsT=ctxn_sb[:, a, 128 * t : 128 * (t + 1)],
                        rhs=wo_sb[:, a, 512 * oc : 512 * (oc + 1)],
                        start=(a == 0),
                        stop=(a == 1),
                    )
                ot = osb.tile([128, 512], F32, tag="o", name="ot")
                if (t + oc) % 2 == 0:
                    nc.vector.tensor_copy(out=ot[:, :], in_=ps[:, :512])
                else:
                    nc.scalar.copy(out=ot[:, :], in_=ps[:, :512])
                nc.sync.dma_start(
                    out=outp[
                        128 * t : 128 * (t + 1),
                        512 * oc : 512 * (oc + 1),
                    ],
                    in_=ot[:, :],
                )

            def _iter():
                with tc.tile_pool(name="misc_ps", bufs=2, space="PSUM") as misc_ps, \
                     tc.tile_pool(name="ctx_ps", bufs=1, space="PSUM") as ctxp, \
                     tc.tile_pool(name="pt", bufs=4) as ptp, \
                     tc.tile_pool(name="zn", bufs=2) as znp, \
                     tc.tile_pool(name="osb", bufs=4) as osb:
                    def _sink(*aps):
                        # consume tensors cheaply so partial builds execute
                        acc = osb.tile([128, 16], F32, tag="sink", name="sink")
                        for idx, ap in enumerate(aps):
                            nc.vector.tensor_copy(
                                out=acc[: ap.shape[0], idx : idx + 1],
                                in_=ap,
                            )
                        nc.sync.dma_start(
                            out=outp[0:128, 0 : len(aps)], in_=acc[:, : len(aps)]
                        )

                    _dma_in()
                    if phases == "dma":
                        _sink(*[xt_sb[:, kc, N - 1 : N] for kc in range(8)],
                              wq_sb[:, 7, 255:256], wk_sb[:, 7, 255:256],
                              wv_sb[:, 7, 259:260], wo_sb[:, 1, 1023:1024])
                        return
                    _qk_proj(misc_ps, 0)
                    _v_proj(misc_ps)
                    if phases == "proj":
                        _qk_proj(misc_ps, 1)
                        _sink(qT_sb[:, 0, N - 1 : N], qT_sb[:, 1, N - 1 : N],
                              kT_sb[:, 0, N - 1 : N], kT_sb[:, 1, N - 1 : N],
                              vc_sb[:, 15, 259:260])
                        return

                    # pair 0 with qk-proj(1) units interleaved into its J loop
                    u1 = iter(list(_qk_units(misc_ps, 1)))

                    def hook0(phase, J):
                        u = next(u1, None)
                        if u is not None:
                            u()

                    _attn_pair(misc_ps, ctxp, ptp, znp, 0, hook0)
                    for u in u1:
                        u()

                    if phases == "attn":
                        _attn_pair(misc_ps, ctxp, ptp, znp, 1)
                        _sink(ctxn_sb[:, 0, N - 1 : N], ctxn_sb[:, 1, N - 1 : N])
                        return

                    # pair 1 with final-linear units interleaved once their
                    # ctx chunk (I = t // 4) has been normalized
                    ready = []
                    emitted = []

                    def hook1(phase, J):
                        for I in (2 * phase, 2 * phase + 1):
                            if J == 4 * I + 3:
                                ready.extend(
                                    (t, oc)
                                    for t in range(4 * I, 4 * I + 4)
                                    for oc in range(2)
                                )
                        if J >= 4 and ready:
                            _final_unit(misc_ps, osb, *ready.pop(0))

                    _attn_pair(misc_ps, ctxp, ptp, znp, 1, hook1)
                    for t_oc in ready:
                        _final_unit(misc_ps, osb, *t_oc)

            if loop == 1:
                _iter()
            else:
                with tc.For_i(0, loop, 1):
                    _iter()

    nc.compile()
    return nc


class _Runner:
    """Jitted PJRT executor for the SPMD program (built once per process)."""

    def __init__(self, nc):
        import jax
        from jax.experimental.shard_map import shard_map
        from jax.sharding import Mesh, NamedSharding, PartitionSpec

        from concourse.bass2jax import (
            _bass_exec_p,
            install_neuronx_cc_hook,
            partition_id_tensor,
        )

        install_neuronx_cc_hook()
        self.nc = nc
        self.jax = jax

        in_names, out_names, out_avals = [], [], []
        partition_name = (
            nc.partition_id_tensor.name if nc.partition_id_tensor else None
        )
        for alloc in nc.m.functions[0].allocations:
            if not isinstance(alloc, mybir.MemoryLocationSet):
                continue
            name = alloc.memorylocations[0].name
            if alloc.kind == "ExternalInput":
                if name != partition_name:
                    in_names.append(name)
            elif alloc.kind == "ExternalOutput":
                out_names.append(name)
                out_avals.append(
                    jax.core.ShapedArray(
                        tuple(alloc.tensor_shape), mybir.dt.np(alloc.dtype)
                    )
                )
        self.in_names = list(in_names)
        self.out_names = out_names
        self.out_avals = out_avals
        n_params = len(in_names)
        n_outs = len(out_names)
        all_names = in_names + out_names
        if partition_name is not None:
            all_names = all_names + [partition_name]

        def _body(*args):
            operands = list(args)
            if partition_name is not None:
                operands.append(partition_id_tensor())
            return tuple(
                _bass_exec_p.bind(
                    *operands,
                    out_avals=tuple(out_avals),
                    in_names=tuple(all_names),
                    out_names=tuple(out_names),
                    lowering_input_output_aliases=(),
                    sim_require_finite=True,
                    sim_require_nnan=True,
                    nc=nc,
                )
            )

        devices = jax.devices()[:NC]
        self.mesh = Mesh(np.asarray(devices), ("core",))
        in_specs = (PartitionSpec("core"),) * (n_params + n_outs)
        out_specs = (PartitionSpec("core"),) * n_outs
        self.fn = jax.jit(
            shard_map(
                _body,
                mesh=self.mesh,
                in_specs=in_specs,
                out_specs=out_specs,
                check_rep=False,
            ),
            keep_unused=True,
        )
        self.sharding = NamedSharding(self.mesh, PartitionSpec("core"))

    def prep(self, in_maps):
        """Concatenate per-core inputs along axis 0 and device_put."""
        arrs = []
        for name in self.in_names:
            arrs.append(np.concatenate([m[name] for m in in_maps], axis=0))
        for av in self.out_avals:
            arrs.append(np.zeros((NC * av.shape[0], *av.shape[1:]), av.dtype))
        return [self.jax.device_put(a, self.sharding) for a in arrs]

    def run(self, dev_args):
        out = self.fn(*dev_args)
        self.jax.block_until_ready(out)
        return out

    def run_async(self, dev_args):
        return self.fn(*dev_args)

    def unpack(self, out):
        res = []
        for c in range(NC):
            res.append(
                {
                    name: np.asarray(out[i]).reshape(NC, *self.out_avals[i].shape)[c]
                    for i, name in enumerate(self.out_names)
                }
            )
        return res


_RUNNER = None


def _get_runner():
    global _RUNNER
    if _RUNNER is None:
        _RUNNER = _Runner(build_nc())
    return _RUNNER


def make_in_maps(x, Wq, bq, Wk, bk, Wv, bv, Wo, bo):
    bf = ml_dtypes.bfloat16
    f32 = np.float32
    x = np.asarray(x, f32)
    mask = np.ascontiguousarray(np.triu(np.ones((128, 128), f32))).astype(bf)
    in_maps = []
    for core in range(NC):
        b, g = core // 4, core % 4
        sl = slice(256 * g, 256 * (g + 1))
        wv_cat = np.zeros((D, 260), f32)
        bv_cat = np.zeros((1, 260), f32)
        for h in range(4):
            col = 256 * g + 64 * h
            wv_cat[:, 65 * h : 65 * h + 64] = Wv[:, col : col + 64]
            bv_cat[0, 65 * h : 65 * h + 64] = bv[col : col + 64]
            bv_cat[0, 65 * h + 64] = 1.0
        in_maps.append(
            {
                "xT": np.ascontiguousarray(x[b].T).reshape(8, 128, N).astype(bf),
                "wq": np.ascontiguousarray(Wq[:, sl]).reshape(8, 128, 256).astype(bf),
                "wk": np.ascontiguousarray(Wk[:, sl]).reshape(8, 128, 256).astype(bf),
                "wv": wv_cat.reshape(8, 128, 260).astype(bf),
                "bq": np.ascontiguousarray(np.asarray(bq, f32)[sl].reshape(2, 128).T),
                "bk": np.ascontiguousarray(np.asarray(bk, f32)[sl].reshape(2, 128).T),
                "bvc": bv_cat.astype(bf),
                "wo": np.ascontiguousarray(
                    np.asarray(Wo, f32)[sl].reshape(2, 128, 1024).transpose(1, 0, 2)
                ).astype(bf),
                "mask": mask,
            }
        )
    return in_maps


def combine(results, bo):
    out = np.zeros((2, N, D), np.float32)
    for core in range(NC):
        out[core // 4] += results[core]["out"]
    out += np.asarray(bo, np.float32)[None, None, :]
    return out


def kernel(x, Wq, bq, Wk, bk, Wv, bv, Wo, bo):
    runner = _get_runner()
    in_maps = make_in_maps(x, Wq, bq, Wk, bk, Wv, bv, Wo, bo)
    dev_args = runner.prep(in_maps)
    results = runner.unpack(runner.run(dev_args))
    return combine(results, bo)


# revision 22
# speedup vs baseline: 1.1858x; 1.1427x over previous
"""Multi-head causal attention on 8 trn2 NeuronCores.

Sharding: data-parallel over batch (2) x tensor-parallel over heads (4 per
core, Megatron-style column-split QKV / row-split output projection).
Per-core partial outputs are summed on the host (+ output bias).
"""

import sys

sys.path.insert(0, "/opt/trn_rl_repo")

import ml_dtypes
import numpy as np

import concourse.bass as bass  # noqa: F401  (import keeps bass registered)
import concourse.tile as tile
from concourse import bacc, mybir

BF16 = mybir.dt.bfloat16
F32 = mybir.dt.float32
AF = mybir.ActivationFunctionType

N = 2048  # sequence length
D = 1024  # model dim
NC = 8  # cores


def build_nc(variant="full", loop=1, phases="full"):
    """Build the (SPMD) Bass program run identically on all 8 cores.

    variant: "full" | "nopb" (skip partition_broadcast, copy unnormalized ctx)
    loop: repeat the whole body N times inside the NEFF (timing harness).
    phases: "dma" | "proj" | "attn" | "full" — truncate after that phase
        (debug builds; partial results sunk to the output tensor).
    """
    nc = bacc.Bacc("TRN2", target_bir_lowering=False, debug=False, num_devices=NC)

    xT = nc.declare_dram_parameter("xT", [8, 128, N], BF16, isOutput=False)
    wq = nc.declare_dram_parameter("wq", [8, 128, 256], BF16, isOutput=False)
    wk = nc.declare_dram_parameter("wk", [8, 128, 256], BF16, isOutput=False)
    wv = nc.declare_dram_parameter("wv", [8, 128, 260], BF16, isOutput=False)
    bqp = nc.declare_dram_parameter("bq", [128, 2], F32, isOutput=False)
    bkp = nc.declare_dram_parameter("bk", [128, 2], F32, isOutput=False)
    bvcp = nc.declare_dram_parameter("bvc", [1, 260], BF16, isOutput=False)
    wo = nc.declare_dram_parameter("wo", [128, 2, 1024], BF16, isOutput=False)
    maskp = nc.declare_dram_parameter("mask", [128, 128], BF16, isOutput=False)
    outp = nc.declare_dram_parameter("out", [N, 1024], F32, isOutput=True)

    with tile.TileContext(nc) as tc:
        with tc.tile_pool(name="singles", bufs=1) as singles:
            xt_sb = singles.tile([128, 8, N], BF16)
            wq_sb = singles.tile([128, 8, 256], BF16)
            wk_sb = singles.tile([128, 8, 256], BF16)
            wv_sb = singles.tile([128, 8, 260], BF16)
            bq_sb = singles.tile([128, 2], F32)
            bk_sb = singles.tile([128, 2], F32)
            bvc_sb = singles.tile([1, 260], BF16)
            wo_sb = singles.tile([128, 2, 1024], BF16)
            mask_sb = singles.tile([128, 128], BF16)
            ones_sb = singles.tile([1, 128], BF16)
            qT_sb = singles.tile([128, 2, N], BF16)
            kT_sb = singles.tile([128, 2, N], BF16)
            vc_sb = singles.tile([128, 16, 260], BF16)
            ctxn_sb = singles.tile([128, 2, N], BF16)

            def _dma_in():
                nc.vector.memset(ones_sb[:, :], 1.0)
                # weights on the SWDGE path, activations on HWDGE — parallel
                # issue queues; one large strided DMA per tensor.
                nc.gpsimd.dma_start(
                    out=wq_sb[:, :, :], in_=wq[:, :, :].rearrange("k p n -> p k n")
                )
                nc.gpsimd.dma_start(
                    out=wk_sb[:, :, :], in_=wk[:, :, :].rearrange("k p n -> p k n")
                )
                nc.gpsimd.dma_start(
                    out=wv_sb[:, :, :], in_=wv[:, :, :].rearrange("k p n -> p k n")
                )
                nc.gpsimd.dma_start(out=bq_sb[:, :], in_=bqp[:, :])
                nc.gpsimd.dma_start(out=bk_sb[:, :], in_=bkp[:, :])
                nc.gpsimd.dma_start(out=bvc_sb[:, :], in_=bvcp[:, :])
                nc.gpsimd.dma_start(out=wo_sb[:, :, :], in_=wo[:, :, :])
                nc.gpsimd.dma_start(out=mask_sb[:, :], in_=maskp[:, :])
                nc.sync.dma_start(out=xt_sb[:, 0, :], in_=xT[0])
                nc.sync.dma_start(out=xt_sb[:, 1, :], in_=xT[1])
                for half in range(3):
                    k0 = 2 * half + 2
                    nc.sync.dma_start(
                        out=xt_sb[:, k0 : k0 + 2, :],
                        in_=xT[k0 : k0 + 2, :, :].rearrange("k p n -> p k n"),
                    )

            def _qk_unit(misc_ps, w_sb, b_sb, o_sb, c, I):
                ps = misc_ps.tile([128, 1024], F32, tag="sc", name="qkps")
                for kc in range(8):
                    nc.tensor.matmul(
                        ps[:, :512],
                        lhsT=w_sb[:, kc, 128 * c : 128 * (c + 1)],
                        rhs=xt_sb[:, kc, 512 * I : 512 * (I + 1)],
                        start=(kc == 0),
                        stop=(kc == 7),
                    )
                nc.vector.tensor_scalar_add(
                    o_sb[:, c, 512 * I : 512 * (I + 1)],
                    ps[:, :512],
                    b_sb[:, c : c + 1],
                )

            def _qk_units(misc_ps, c):
                for w_sb, b_sb, o_sb in (
                    (wq_sb, bq_sb, qT_sb),
                    (wk_sb, bk_sb, kT_sb),
                ):
                    for I in range(4):
                        yield lambda w=w_sb, b=b_sb, o=o_sb, i=I: _qk_unit(
                            misc_ps, w, b, o, c, i
                        )

            def _qk_proj(misc_ps, c):
                for u in _qk_units(misc_ps, c):
                    u()

            def _v_proj(misc_ps):
                for J in range(16):
                    ps = misc_ps.tile([128, 1024], F32, tag="sc", name="vps")
                    for kc in range(8):
                        nc.tensor.matmul(
                            ps[:, :260],
                            lhsT=xt_sb[:, kc, 128 * J : 128 * (J + 1)],
                            rhs=wv_sb[:, kc, :],
                            start=(kc == 0),
                            stop=False,
                        )
                    nc.tensor.matmul(
                        ps[:, :260],
                        lhsT=ones_sb[:, :],
                        rhs=bvc_sb[:, :],
                        start=False,
                        stop=True,
                    )
                    nc.vector.tensor_copy(out=vc_sb[:, J, :], in_=ps[:, :260])

            def _norm_chunk(znp, h, I, ctx_tile):
                c, po = h // 2, 64 * (h % 2)
                if variant == "nopb":
                    nc.vector.tensor_copy(
                        out=ctxn_sb[po : po + 64, c, 512 * I : 512 * (I + 1)],
                        in_=ctx_tile[0:64, :],
                    )
                    return
                zr = znp.tile([1, 512], F32, tag="zr", name="zr")
                nc.vector.reciprocal(zr[:, :], ctx_tile[64:65, :])
                zb = znp.tile([64, 512], F32, tag="zb", name="zb")
                nc.gpsimd.partition_broadcast(zb[:, :], zr[:, :], channels=64)
                nc.vector.tensor_mul(
                    ctxn_sb[po : po + 64, c, 512 * I : 512 * (I + 1)],
                    ctx_tile[0:64, :],
                    zb[:, :],
                )

            def _attn_pair(misc_ps, ctxp, ptp, znp, p, hook=None):
                """Heads (2p, 2p+1) together: even head at partitions 0-63,
                odd at 64-127 -> row-disjoint tile_positions let the PE run
                both K=64 score matmuls concurrently.

                i advances in 512-wide windows (4 phases).  Per (phase, J):
                one [128, 1024] PSUM tile holds both heads' scores, one exp
                covers both, and the ctx matmuls are emitted one step late so
                the in-order PE queue always has the next scores (plus
                `hook(phase, J)` filler) to run while ACT computes exp."""
                c = p

                def _emit_ctx(phase, J, pt, L, gs0, ctx_t):
                    lo = gs0 - 512 * phase
                    for hh in range(2):
                        h = 2 * p + hh
                        nc.tensor.matmul(
                            ctx_t[hh][:, lo : lo + L],
                            lhsT=vc_sb[:, J, 65 * h : 65 * h + 65],
                            rhs=pt[:, 512 * hh : 512 * hh + L],
                            start=(J == 0),
                            stop=(J == 4 * phase + 3),
                        )

                for phase in range(4):
                    i0 = 512 * phase
                    ctx_t = {
                        hh: ctxp.tile(
                            [65, 512], F32, name=f"ctx{hh}", tag=f"ctx{hh}"
                        )
                        for hh in range(2)
                    }
                    pend = None
                    for J in range(4 * phase + 4):
                        gs0 = max(i0, 128 * J)
                        L = i0 + 512 - gs0
                        ps = misc_ps.tile([128, 1024], F32, tag="sc", name="scps")
                        pt = ptp.tile([128, 1024], BF16, tag="pt", name="pt")
                        for hh in range(2):
                            po = 64 * hh
                            nc.tensor.matmul(
                                ps[:, 512 * hh : 512 * hh + L],
                                lhsT=kT_sb[
                                    po : po + 64, c, 128 * J : 128 * (J + 1)
                                ],
                                rhs=qT_sb[po : po + 64, c, gs0 : gs0 + L],
                                start=True,
                                stop=True,
                            )
                        if pend is not None:
                            _emit_ctx(*pend)
                            pend = None
                        if L == 512:
                            nc.scalar.activation(
                                pt[:, :], ps[:, :], AF.Exp, scale=0.125
                            )
                        else:
                            nc.scalar.activation(
                                pt[:, :L], ps[:, :L], AF.Exp, scale=0.125
                            )
                            nc.scalar.activation(
                                pt[:, 512 : 512 + L],
                                ps[:, 512 : 512 + L],
                                AF.Exp,
                                scale=0.125,
                            )
                        if J >= 4 * phase and variant != "nomask":
                            nc.vector.tensor_mul(
                                pt[:, :128], pt[:, :128], mask_sb[:, :]
                            )
                            nc.vector.tensor_mul(
                                pt[:, 512:640], pt[:, 512:640], mask_sb[:, :]
                            )
                        if variant == "noctx":
                            nc.vector.tensor_copy(
                                out=ctxn_sb[0:128, c, J : J + 1], in_=pt[:, 0:1]
                            )
                        else:
                            pend = (phase, J, pt, L, gs0, ctx_t)
                        if hook is not None:
                            hook(phase, J)
                    if pend is not None:
                        _emit_ctx(*pend)
                    if variant != "noctx":
                        for hh in range(2):
                            _norm_chunk(znp, 2 * p + hh, phase, ctx_t[hh])

            def _final_unit(misc_ps, osb, t, oc):
                ps = misc_ps.tile([128, 1024], F32, tag="sc", name="fps")
                for a in range(2):
                    nc.tensor.matmul(
                        ps[:, :512],
                        lhsT=ctxn_sb[:, a, 128 * t : 128 * (t + 1)],
                        rhs=wo_sb[:, a, 512 * oc : 512 * (oc + 1)],
                        start=(a == 0),
                        stop=(a == 1),
                    )
                ot = osb.tile([128, 512], F32, tag="o", name="ot")
                if (t + oc) % 2 == 0:
                    nc.vector.tensor_copy(out=ot[:, :], in_=ps[:, :512])
                else:
                    nc.scalar.copy(out=ot[:, :], in_=ps[:, :512])
                nc.sync.dma_start(
                    out=outp[
                        128 * t : 128 * (t + 1),
                        512 * oc : 512 * (oc + 1),
                    ],
                    in_=ot[:, :],
                )

            def _iter():
                with tc.tile_pool(name="misc_ps", bufs=2, space="PSUM") as misc_ps, \
                     tc.tile_pool(name="ctx_ps", bufs=1, space="PSUM") as ctxp, \
                     tc.tile_pool(name="pt", bufs=4) as ptp, \
                     tc.tile_pool(name="zn", bufs=2) as znp, \
                     tc.tile_pool(name="osb", bufs=4) as osb:
                    def _sink(*aps):
                        # consume tensors cheaply so partial builds execute
                        acc = osb.tile([128, 16], F32, tag="sink", name="sink")
                        for idx, ap in enumerate(aps):
                            nc.vector.tensor_copy(
                                out=acc[: ap.shape[0], idx : idx + 1],
                                in_=ap,
                            )
                        nc.sync.dma_start(
                            out=outp[0:128, 0 : len(aps)], in_=acc[:, : len(aps)]
                        )

                    _dma_in()
                    if phases == "dma":
                        _sink(*[xt_sb[:, kc, N - 1 : N] for kc in range(8)],
                              wq_sb[:, 7, 255:256], wk_sb[:, 7, 255:256],
                              wv_sb[:, 7, 259:260], wo_sb[:, 1, 1023:1024])
                        return
                    _qk_proj(misc_ps, 0)
                    _v_proj(misc_ps)
                    if phases == "proj":
                        _qk_proj(misc_ps, 1)
                        _sink(qT_sb[:, 0, N - 1 : N], qT_sb[:, 1, N - 1 : N],
                              kT_sb[:, 0, N - 1 : N], kT_sb[:, 1, N - 1 : N],
                              vc_sb[:, 15, 259:260])
                        return

                    # pair 0 with qk-proj(1) units interleaved into its J loop
                    u1 = iter(list(_qk_units(misc_ps, 1)))

                    def hook0(phase, J):
                        u = next(u1, None)
                        if u is not None:
                            u()

                    _attn_pair(misc_ps, ctxp, ptp, znp, 0, hook0)
                    for u in u1:
                        u()

                    if phases == "attn":
                        _attn_pair(misc_ps, ctxp, ptp, znp, 1)
                        _sink(ctxn_sb[:, 0, N - 1 : N], ctxn_sb[:, 1, N - 1 : N])
                        return

                    # pair 1 with final-linear units interleaved once their
                    # ctx chunk (I = t // 4 = phase) has been normalized
                    ready = []
                    unlocked = set()

                    def hook1(phase, J):
                        for I in range(phase):
                            if I not in unlocked:
                                unlocked.add(I)
                                ready.extend(
                                    (t, oc)
                                    for t in range(4 * I, 4 * I + 4)
                                    for oc in range(2)
                                )
                        if ready:
                            _final_unit(misc_ps, osb, *ready.pop(0))

                    _attn_pair(misc_ps, ctxp, ptp, znp, 1, hook1)
                    for I in range(4):
                        if I not in unlocked:
                            ready.extend(
                                (t, oc)
                                for t in range(4 * I, 4 * I + 4)
                                for oc in range(2)
                            )
                    for t_oc in ready:
                        _final_unit(misc_ps, osb, *t_oc)

            if loop == 1:
                _iter()
            else:
                with tc.For_i(0, loop, 1):
                    _iter()

    nc.compile()
    return nc


class _Runner:
    """Jitted PJRT executor for the SPMD program (built once per process)."""

    def __init__(self, nc):
        import jax
        from jax.experimental.shard_map import shard_map
        from jax.sharding import Mesh, NamedSharding, PartitionSpec

        from concourse.bass2jax import (
            _bass_exec_p,
            install_neuronx_cc_hook,
            partition_id_tensor,
        )

        install_neuronx_cc_hook()
        self.nc = nc
        self.jax = jax

        in_names, out_names, out_avals = [], [], []
        partition_name = (
            nc.partition_id_tensor.name if nc.partition_id_tensor else None
        )
        for alloc in nc.m.functions[0].allocations:
            if not isinstance(alloc, mybir.MemoryLocationSet):
                continue
            name = alloc.memorylocations[0].name
            if alloc.kind == "ExternalInput":
                if name != partition_name:
                    in_names.append(name)
            elif alloc.kind == "ExternalOutput":
                out_names.append(name)
                out_avals.append(
                    jax.core.ShapedArray(
                        tuple(alloc.tensor_shape), mybir.dt.np(alloc.dtype)
                    )
                )
        self.in_names = list(in_names)
        self.out_names = out_names
        self.out_avals = out_avals
        n_params = len(in_names)
        n_outs = len(out_names)
        all_names = in_names + out_names
        if partition_name is not None:
            all_names = all_names + [partition_name]

        def _body(*args):
            operands = list(args)
            if partition_name is not None:
                operands.append(partition_id_tensor())
            return tuple(
                _bass_exec_p.bind(
                    *operands,
                    out_avals=tuple(out_avals),
                    in_names=tuple(all_names),
                    out_names=tuple(out_names),
                    lowering_input_output_aliases=(),
                    sim_require_finite=True,
                    sim_require_nnan=True,
                    nc=nc,
                )
            )

        devices = jax.devices()[:NC]
        self.mesh = Mesh(np.asarray(devices), ("core",))
        in_specs = (PartitionSpec("core"),) * (n_params + n_outs)
        out_specs = (PartitionSpec("core"),) * n_outs
        self.fn = jax.jit(
            shard_map(
                _body,
                mesh=self.mesh,
                in_specs=in_specs,
                out_specs=out_specs,
                check_rep=False,
            ),
            keep_unused=True,
        )
        self.sharding = NamedSharding(self.mesh, PartitionSpec("core"))

    def prep(self, in_maps):
        """Concatenate per-core inputs along axis 0 and device_put."""
        arrs = []
        for name in self.in_names:
            arrs.append(np.concatenate([m[name] for m in in_maps], axis=0))
        for av in self.out_avals:
            arrs.append(np.zeros((NC * av.shape[0], *av.shape[1:]), av.dtype))
        return [self.jax.device_put(a, self.sharding) for a in arrs]

    def run(self, dev_args):
        out = self.fn(*dev_args)
        self.jax.block_until_ready(out)
        return out

    def run_async(self, dev_args):
        return self.fn(*dev_args)

    def unpack(self, out):
        res = []
        for c in range(NC):
            res.append(
                {
                    name: np.asarray(out[i]).reshape(NC, *self.out_avals[i].shape)[c]
                    for i, name in enumerate(self.out_names)
                }
            )
        return res


_RUNNER = None


def _get_runner():
    global _RUNNER
    if _RUNNER is None:
        _RUNNER = _Runner(build_nc())
    return _RUNNER


def make_in_maps(x, Wq, bq, Wk, bk, Wv, bv, Wo, bo):
    bf = ml_dtypes.bfloat16
    f32 = np.float32
    x = np.asarray(x, f32)
    mask = np.ascontiguousarray(np.triu(np.ones((128, 128), f32))).astype(bf)
    in_maps = []
    for core in range(NC):
        b, g = core // 4, core % 4
        sl = slice(256 * g, 256 * (g + 1))
        wv_cat = np.zeros((D, 260), f32)
        bv_cat = np.zeros((1, 260), f32)
        for h in range(4):
            col = 256 * g + 64 * h
            wv_cat[:, 65 * h : 65 * h + 64] = Wv[:, col : col + 64]
            bv_cat[0, 65 * h : 65 * h + 64] = bv[col : col + 64]
            bv_cat[0, 65 * h + 64] = 1.0
        in_maps.append(
            {
                "xT": np.ascontiguousarray(x[b].T).reshape(8, 128, N).astype(bf),
                "wq": np.ascontiguousarray(Wq[:, sl]).reshape(8, 128, 256).astype(bf),
                "wk": np.ascontiguousarray(Wk[:, sl]).reshape(8, 128, 256).astype(bf),
                "wv": wv_cat.reshape(8, 128, 260).astype(bf),
                "bq": np.ascontiguousarray(np.asarray(bq, f32)[sl].reshape(2, 128).T),
                "bk": np.ascontiguousarray(np.asarray(bk, f32)[sl].reshape(2, 128).T),
                "bvc": bv_cat.astype(bf),
                "wo": np.ascontiguousarray(
                    np.asarray(Wo, f32)[sl].reshape(2, 128, 1024).transpose(1, 0, 2)
                ).astype(bf),
                "mask": mask,
            }
        )
    return in_maps


def combine(results, bo):
    out = np.zeros((2, N, D), np.float32)
    for core in range(NC):
        out[core // 4] += results[core]["out"]
    out += np.asarray(bo, np.float32)[None, None, :]
    return out


def kernel(x, Wq, bq, Wk, bk, Wv, bv, Wo, bo):
    runner = _get_runner()
    in_maps = make_in_maps(x, Wq, bq, Wk, bk, Wv, bv, Wo, bo)
    dev_args = runner.prep(in_maps)
    results = runner.unpack(runner.run(dev_args))
    return combine(results, bo)


# revision 26
# speedup vs baseline: 1.1966x; 1.0091x over previous
"""Multi-head causal attention on 8 trn2 NeuronCores.

Sharding: data-parallel over batch (2) x tensor-parallel over heads (4 per
core, Megatron-style column-split QKV / row-split output projection).
Per-core partial outputs are summed on the host (+ output bias).
"""

import sys

sys.path.insert(0, "/opt/trn_rl_repo")

import ml_dtypes
import numpy as np

import concourse.bass as bass  # noqa: F401  (import keeps bass registered)
import concourse.tile as tile
from concourse import bacc, mybir

BF16 = mybir.dt.bfloat16
F32 = mybir.dt.float32
AF = mybir.ActivationFunctionType

N = 2048  # sequence length
D = 1024  # model dim
NC = 8  # cores


def build_nc(variant="full", loop=1, phases="full"):
    """Build the (SPMD) Bass program run identically on all 8 cores.

    variant: "full" | "nopb" (skip partition_broadcast, copy unnormalized ctx)
    loop: repeat the whole body N times inside the NEFF (timing harness).
    phases: "dma" | "proj" | "attn" | "full" — truncate after that phase
        (debug builds; partial results sunk to the output tensor).
    """
    nc = bacc.Bacc("TRN2", target_bir_lowering=False, debug=False, num_devices=NC)

    xT = nc.declare_dram_parameter("xT", [8, 128, N], BF16, isOutput=False)
    wq = nc.declare_dram_parameter("wq", [8, 128, 256], BF16, isOutput=False)
    wk = nc.declare_dram_parameter("wk", [8, 128, 256], BF16, isOutput=False)
    wv = nc.declare_dram_parameter("wv", [8, 128, 260], BF16, isOutput=False)
    bqp = nc.declare_dram_parameter("bq", [128, 2], F32, isOutput=False)
    bkp = nc.declare_dram_parameter("bk", [128, 2], F32, isOutput=False)
    bvcp = nc.declare_dram_parameter("bvc", [1, 260], BF16, isOutput=False)
    wo = nc.declare_dram_parameter("wo", [128, 2, 1024], BF16, isOutput=False)
    maskp = nc.declare_dram_parameter("mask", [128, 128], BF16, isOutput=False)
    outp = nc.declare_dram_parameter("out", [N, 1024], F32, isOutput=True)

    with tile.TileContext(nc) as tc:
        with tc.tile_pool(name="singles", bufs=1) as singles:
            xt_sb = singles.tile([128, 8, N], BF16)
            wq_sb = singles.tile([128, 8, 256], BF16)
            wk_sb = singles.tile([128, 8, 256], BF16)
            wv_sb = singles.tile([128, 8, 260], BF16)
            bq_sb = singles.tile([128, 2], F32)
            bk_sb = singles.tile([128, 2], F32)
            bvc_sb = singles.tile([1, 260], BF16)
            wo_sb = singles.tile([128, 2, 1024], BF16)
            mask_sb = singles.tile([128, 128], BF16)
            ones_sb = singles.tile([1, 128], BF16)
            qT_sb = singles.tile([128, 2, N], BF16)
            kT_sb = singles.tile([128, 2, N], BF16)
            vc_sb = singles.tile([128, 16, 260], BF16)
            ctxn_sb = singles.tile([128, 2, N], BF16)

            def _dma_in():
                nc.vector.memset(ones_sb[:, :], 1.0)
                # weights on the SWDGE path, activations on HWDGE — parallel
                # issue queues; one large strided DMA per tensor.
                nc.gpsimd.dma_start(
                    out=wq_sb[:, :, :], in_=wq[:, :, :].rearrange("k p n -> p k n")
                )
                nc.gpsimd.dma_start(
                    out=wk_sb[:, :, :], in_=wk[:, :, :].rearrange("k p n -> p k n")
                )
                nc.gpsimd.dma_start(
                    out=wv_sb[:, :, :], in_=wv[:, :, :].rearrange("k p n -> p k n")
                )
                nc.gpsimd.dma_start(out=bq_sb[:, :], in_=bqp[:, :])
                nc.gpsimd.dma_start(out=bk_sb[:, :], in_=bkp[:, :])
                nc.gpsimd.dma_start(out=bvc_sb[:, :], in_=bvcp[:, :])
                nc.gpsimd.dma_start(out=wo_sb[:, :, :], in_=wo[:, :, :])
                nc.gpsimd.dma_start(out=mask_sb[:, :], in_=maskp[:, :])
                nc.sync.dma_start(out=xt_sb[:, 0, :], in_=xT[0])
                nc.sync.dma_start(out=xt_sb[:, 1, :], in_=xT[1])
                for half in range(3):
                    k0 = 2 * half + 2
                    nc.sync.dma_start(
                        out=xt_sb[:, k0 : k0 + 2, :],
                        in_=xT[k0 : k0 + 2, :, :].rearrange("k p n -> p k n"),
                    )

            def _qk_unit(misc_ps, w_sb, b_sb, o_sb, c, I):
                ps = misc_ps.tile([128, 1024], F32, tag="sc", name="qkps")
                for kc in range(8):
                    nc.tensor.matmul(
                        ps[:, :512],
                        lhsT=w_sb[:, kc, 128 * c : 128 * (c + 1)],
                        rhs=xt_sb[:, kc, 512 * I : 512 * (I + 1)],
                        start=(kc == 0),
                        stop=(kc == 7),
                    )
                nc.vector.tensor_scalar_add(
                    o_sb[:, c, 512 * I : 512 * (I + 1)],
                    ps[:, :512],
                    b_sb[:, c : c + 1],
                )

            def _qk_units(misc_ps, c):
                for w_sb, b_sb, o_sb in (
                    (wq_sb, bq_sb, qT_sb),
                    (wk_sb, bk_sb, kT_sb),
                ):
                    for I in range(4):
                        yield lambda w=w_sb, b=b_sb, o=o_sb, i=I: _qk_unit(
                            misc_ps, w, b, o, c, i
                        )

            def _qk_proj(misc_ps, c):
                for u in _qk_units(misc_ps, c):
                    u()

            def _v_unit(misc_ps, J):
                ps = misc_ps.tile([128, 1024], F32, tag="sc", name="vps")
                for kc in range(8):
                    nc.tensor.matmul(
                        ps[:, :260],
                        lhsT=xt_sb[:, kc, 128 * J : 128 * (J + 1)],
                        rhs=wv_sb[:, kc, :],
                        start=(kc == 0),
                        stop=False,
                    )
                nc.tensor.matmul(
                    ps[:, :260],
                    lhsT=ones_sb[:, :],
                    rhs=bvc_sb[:, :],
                    start=False,
                    stop=True,
                )
                nc.vector.tensor_copy(out=vc_sb[:, J, :], in_=ps[:, :260])

            def _v_proj(misc_ps):
                for J in range(16):
                    _v_unit(misc_ps, J)

            def _qk_units_by_I(misc_ps, c):
                out = []
                for I in range(4):
                    for w_sb, b_sb, o_sb in (
                        (wq_sb, bq_sb, qT_sb),
                        (wk_sb, bk_sb, kT_sb),
                    ):
                        out.append(
                            lambda w=w_sb, b=b_sb, o=o_sb, i=I: _qk_unit(
                                misc_ps, w, b, o, c, i
                            )
                        )
                return out

            def _norm_chunk(znp, h, I, ctx_tile):
                c, po = h // 2, 64 * (h % 2)
                if variant == "nopb":
                    nc.vector.tensor_copy(
                        out=ctxn_sb[po : po + 64, c, 512 * I : 512 * (I + 1)],
                        in_=ctx_tile[0:64, :],
                    )
                    return
                zr = znp.tile([1, 512], F32, tag="zr", name="zr")
                nc.vector.reciprocal(zr[:, :], ctx_tile[64:65, :])
                zb = znp.tile([64, 512], F32, tag="zb", name="zb")
                nc.gpsimd.partition_broadcast(zb[:, :], zr[:, :], channels=64)
                nc.vector.tensor_mul(
                    ctxn_sb[po : po + 64, c, 512 * I : 512 * (I + 1)],
                    ctx_tile[0:64, :],
                    zb[:, :],
                )

            def _attn_pair(misc_ps, ctxp, ptp, znp, p, hook=None):
                """Heads (2p, 2p+1) together: even head at partitions 0-63,
                odd at 64-127 -> row-disjoint tile_positions let the PE run
                both K=64 score matmuls concurrently.

                i advances in 512-wide windows (4 phases).  Per (phase, J):
                one [128, 1024] PSUM tile holds both heads' scores, one exp
                covers both, and the ctx matmuls are emitted one step late so
                the in-order PE queue always has the next scores (plus
                `hook(phase, J)` filler) to run while ACT computes exp."""
                c = p

                def _emit_ctx(phase, J, pt, L, gs0, ctx_t):
                    lo = gs0 - 512 * phase
                    for hh in range(2):
                        h = 2 * p + hh
                        nc.tensor.matmul(
                            ctx_t[hh][:, lo : lo + L],
                            lhsT=vc_sb[:, J, 65 * h : 65 * h + 65],
                            rhs=pt[:, 512 * hh : 512 * hh + L],
                            start=(J == 0),
                            stop=(J == 4 * phase + 3),
                        )

                for phase in range(4):
                    i0 = 512 * phase
                    ctx_t = {
                        hh: ctxp.tile(
                            [65, 512], F32, name=f"ctx{hh}", tag=f"ctx{hh}"
                        )
                        for hh in range(2)
                    }
                    pend = None
                    for J in range(4 * phase + 4):
                        gs0 = max(i0, 128 * J)
                        L = i0 + 512 - gs0
                        ps = misc_ps.tile([128, 1024], F32, tag="sc", name="scps")
                        pt = ptp.tile([128, 1024], BF16, tag="pt", name="pt")
                        for hh in range(2):
                            po = 64 * hh
                            nc.tensor.matmul(
                                ps[:, 512 * hh : 512 * hh + L],
                                lhsT=kT_sb[
                                    po : po + 64, c, 128 * J : 128 * (J + 1)
                                ],
                                rhs=qT_sb[po : po + 64, c, gs0 : gs0 + L],
                                start=True,
                                stop=True,
                            )
                        if pend is not None:
                            _emit_ctx(*pend)
                            pend = None
                        if L == 512:
                            nc.scalar.activation(
                                pt[:, :], ps[:, :], AF.Exp, scale=0.125
                            )
                        else:
                            nc.scalar.activation(
                                pt[:, :L], ps[:, :L], AF.Exp, scale=0.125
                            )
                            nc.scalar.activation(
                                pt[:, 512 : 512 + L],
                                ps[:, 512 : 512 + L],
                                AF.Exp,
                                scale=0.125,
                            )
                        if J >= 4 * phase and variant != "nomask":
                            nc.vector.tensor_mul(
                                pt[:, :128], pt[:, :128], mask_sb[:, :]
                            )
                            nc.vector.tensor_mul(
                                pt[:, 512:640], pt[:, 512:640], mask_sb[:, :]
                            )
                        if variant == "noctx":
                            nc.vector.tensor_copy(
                                out=ctxn_sb[0:128, c, J : J + 1], in_=pt[:, 0:1]
                            )
                        else:
                            pend = (phase, J, pt, L, gs0, ctx_t)
                        if hook is not None:
                            hook(phase, J)
                    if pend is not None:
                        _emit_ctx(*pend)
                    if variant != "noctx":
                        for hh in range(2):
                            _norm_chunk(znp, 2 * p + hh, phase, ctx_t[hh])
                    if hook is not None:
                        hook(phase, None)  # end of phase: drain fillers

            def _final_unit(misc_ps, osb, t, oc):
                ps = misc_ps.tile([128, 1024], F32, tag="sc", name="fps")
                for a in range(2):
                    nc.tensor.matmul(
                        ps[:, :512],
                        lhsT=ctxn_sb[:, a, 128 * t : 128 * (t + 1)],
                        rhs=wo_sb[:, a, 512 * oc : 512 * (oc + 1)],
                        start=(a == 0),
                        stop=(a == 1),
                    )
                ot = osb.tile([128, 512], F32, tag="o", name="ot")
                if (t + oc) % 2 == 0:
                    nc.vector.tensor_copy(out=ot[:, :], in_=ps[:, :512])
                else:
                    nc.scalar.copy(out=ot[:, :], in_=ps[:, :512])
                nc.sync.dma_start(
                    out=outp[
                        128 * t : 128 * (t + 1),
                        512 * oc : 512 * (oc + 1),
                    ],
                    in_=ot[:, :],
                )

            def _iter():
                with tc.tile_pool(name="misc_ps", bufs=2, space="PSUM") as misc_ps, \
                     tc.tile_pool(name="ctx_ps", bufs=1, space="PSUM") as ctxp, \
                     tc.tile_pool(name="pt", bufs=4) as ptp, \
                     tc.tile_pool(name="zn", bufs=2) as znp, \
                     tc.tile_pool(name="osb", bufs=4) as osb:
                    def _sink(*aps):
                        # consume tensors cheaply so partial builds execute
                        acc = osb.tile([128, 16], F32, tag="sink", name="sink")
                        for idx, ap in enumerate(aps):
                            nc.vector.tensor_copy(
                                out=acc[: ap.shape[0], idx : idx + 1],
                                in_=ap,
                            )
                        nc.sync.dma_start(
                            out=outp[0:128, 0 : len(aps)], in_=acc[:, : len(aps)]
                        )

                    _dma_in()
                    if phases == "dma":
                        _sink(*[xt_sb[:, kc, N - 1 : N] for kc in range(8)],
                              wq_sb[:, 7, 255:256], wk_sb[:, 7, 255:256],
                              wv_sb[:, 7, 259:260], wo_sb[:, 1, 1023:1024])
                        return
                    if phases == "proj":
                        _qk_proj(misc_ps, 0)
                        _v_proj(misc_ps)
                        _qk_proj(misc_ps, 1)
                        _sink(qT_sb[:, 0, N - 1 : N], qT_sb[:, 1, N - 1 : N],
                              kT_sb[:, 0, N - 1 : N], kT_sb[:, 1, N - 1 : N],
                              vc_sb[:, 15, 259:260])
                        return

                    # Fused ramp: emit just enough qk/v projection work for
                    # pair-0 phase 0, then feed the rest (and all of qk c=1)
                    # into pair-0's attention chain as per-step fillers.
                    qk0 = _qk_units_by_I(misc_ps, 0)
                    qk1 = _qk_units_by_I(misc_ps, 1)
                    vu = [lambda J=J: _v_unit(misc_ps, J) for J in range(16)]

                    for u in qk0[0:2] + vu[0:4]:
                        u()
                    fill0 = {
                        0: qk0[2:4] + vu[4:6],
                        1: vu[6:8] + qk0[4:6] + vu[8:12],
                        2: qk0[6:8] + vu[12:16] + qk1[0:6],
                        3: qk1[6:16],
                    }

                    def hook0(phase, J):
                        lst = fill0.get(phase)
                        if not lst:
                            return
                        if J is None:
                            while lst:
                                lst.pop(0)()
                        else:
                            lst.pop(0)()

                    _attn_pair(misc_ps, ctxp, ptp, znp, 0, hook0)

                    if phases == "attn":
                        _attn_pair(misc_ps, ctxp, ptp, znp, 1)
                        _sink(ctxn_sb[:, 0, N - 1 : N], ctxn_sb[:, 1, N - 1 : N])
                        return

                    # pair 1 with final-linear units interleaved once their
                    # ctx chunk (I = t // 4 = phase) has been normalized
                    ready = []
                    unlocked = set()

                    def hook1(phase, J):
                        if J is None:
                            return
                        for I in range(phase):
                            if I not in unlocked:
                                unlocked.add(I)
                                ready.extend(
                                    (t, oc)
                                    for t in range(4 * I, 4 * I + 4)
                                    for oc in range(2)
                                )
                        if ready:
                            _final_unit(misc_ps, osb, *ready.pop(0))

                    _attn_pair(misc_ps, ctxp, ptp, znp, 1, hook1)
                    for I in range(4):
                        if I not in unlocked:
                            ready.extend(
                                (t, oc)
                                for t in range(4 * I, 4 * I + 4)
                                for oc in range(2)
                            )
                    for t_oc in ready:
                        _final_unit(misc_ps, osb, *t_oc)

            if loop == 1:
                _iter()
            else:
                with tc.For_i(0, loop, 1):
                    _iter()

    nc.compile()
    return nc


class _Runner:
    """Jitted PJRT executor for the SPMD program (built once per process)."""

    def __init__(self, nc):
        import jax
        from jax.experimental.shard_map import shard_map
        from jax.sharding import Mesh, NamedSharding, PartitionSpec

        from concourse.bass2jax import (
            _bass_exec_p,
            install_neuronx_cc_hook,
            partition_id_tensor,
        )

        install_neuronx_cc_hook()
        self.nc = nc
        self.jax = jax

        in_names, out_names, out_avals = [], [], []
        partition_name = (
            nc.partition_id_tensor.name if nc.partition_id_tensor else None
        )
        for alloc in nc.m.functions[0].allocations:
            if not isinstance(alloc, mybir.MemoryLocationSet):
                continue
            name = alloc.memorylocations[0].name
            if alloc.kind == "ExternalInput":
                if name != partition_name:
                    in_names.append(name)
            elif alloc.kind == "ExternalOutput":
                out_names.append(name)
                out_avals.append(
                    jax.core.ShapedArray(
                        tuple(alloc.tensor_shape), mybir.dt.np(alloc.dtype)
                    )
                )
        self.in_names = list(in_names)
        self.out_names = out_names
        self.out_avals = out_avals
        n_params = len(in_names)
        n_outs = len(out_names)
        all_names = in_names + out_names
        if partition_name is not None:
            all_names = all_names + [partition_name]

        def _body(*args):
            operands = list(args)
            if partition_name is not None:
                operands.append(partition_id_tensor())
            return tuple(
                _bass_exec_p.bind(
                    *operands,
                    out_avals=tuple(out_avals),
                    in_names=tuple(all_names),
                    out_names=tuple(out_names),
                    lowering_input_output_aliases=(),
                    sim_require_finite=True,
                    sim_require_nnan=True,
                    nc=nc,
                )
            )

        devices = jax.devices()[:NC]
        self.mesh = Mesh(np.asarray(devices), ("core",))
        in_specs = (PartitionSpec("core"),) * (n_params + n_outs)
        out_specs = (PartitionSpec("core"),) * n_outs
        self.fn = jax.jit(
            shard_map(
                _body,
                mesh=self.mesh,
                in_specs=in_specs,
                out_specs=out_specs,
                check_rep=False,
            ),
            keep_unused=True,
        )
        self.sharding = NamedSharding(self.mesh, PartitionSpec("core"))

    def prep(self, in_maps):
        """Concatenate per-core inputs along axis 0 and device_put."""
        arrs = []
        for name in self.in_names:
            arrs.append(np.concatenate([m[name] for m in in_maps], axis=0))
        for av in self.out_avals:
            arrs.append(np.zeros((NC * av.shape[0], *av.shape[1:]), av.dtype))
        return [self.jax.device_put(a, self.sharding) for a in arrs]

    def run(self, dev_args):
        out = self.fn(*dev_args)
        self.jax.block_until_ready(out)
        return out

    def run_async(self, dev_args):
        return self.fn(*dev_args)

    def unpack(self, out):
        res = []
        for c in range(NC):
            res.append(
                {
                    name: np.asarray(out[i]).reshape(NC, *self.out_avals[i].shape)[c]
                    for i, name in enumerate(self.out_names)
                }
            )
        return res


_RUNNER = None


def _get_runner():
    global _RUNNER
    if _RUNNER is None:
        _RUNNER = _Runner(build_nc())
    return _RUNNER


def make_in_maps(x, Wq, bq, Wk, bk, Wv, bv, Wo, bo):
    bf = ml_dtypes.bfloat16
    f32 = np.float32
    x = np.asarray(x, f32)
    mask = np.ascontiguousarray(np.triu(np.ones((128, 128), f32))).astype(bf)
    in_maps = []
    for core in range(NC):
        b, g = core // 4, core % 4
        sl = slice(256 * g, 256 * (g + 1))
        wv_cat = np.zeros((D, 260), f32)
        bv_cat = np.zeros((1, 260), f32)
        for h in range(4):
            col = 256 * g + 64 * h
            wv_cat[:, 65 * h : 65 * h + 64] = Wv[:, col : col + 64]
            bv_cat[0, 65 * h : 65 * h + 64] = bv[col : col + 64]
            bv_cat[0, 65 * h + 64] = 1.0
        in_maps.append(
            {
                "xT": np.ascontiguousarray(x[b].T).reshape(8, 128, N).astype(bf),
                "wq": np.ascontiguousarray(Wq[:, sl]).reshape(8, 128, 256).astype(bf),
                "wk": np.ascontiguousarray(Wk[:, sl]).reshape(8, 128, 256).astype(bf),
                "wv": wv_cat.reshape(8, 128, 260).astype(bf),
                "bq": np.ascontiguousarray(np.asarray(bq, f32)[sl].reshape(2, 128).T),
                "bk": np.ascontiguousarray(np.asarray(bk, f32)[sl].reshape(2, 128).T),
                "bvc": bv_cat.astype(bf),
                "wo": np.ascontiguousarray(
                    np.asarray(Wo, f32)[sl].reshape(2, 128, 1024).transpose(1, 0, 2)
                ).astype(bf),
                "mask": mask,
            }
        )
    return in_maps


def combine(results, bo):
    out = np.zeros((2, N, D), np.float32)
    for core in range(NC):
        out[core // 4] += results[core]["out"]
    out += np.asarray(bo, np.float32)[None, None, :]
    return out


def kernel(x, Wq, bq, Wk, bk, Wv, bv, Wo, bo):
    runner = _get_runner()
    in_maps = make_in_maps(x, Wq, bq, Wk, bk, Wv, bv, Wo, bo)
    dev_args = runner.prep(in_maps)
    results = runner.unpack(runner.run(dev_args))
    return combine(results, bo)
